# revision 8
# baseline (speedup 1.0000x reference)
"""Trainium2 Bass kernel for nn_CELoss_4896262717859.

Computes, for each query column c = idx_node[k] of a sparse adjacency matrix
(diagonal zeroed), a cross-entropy-style loss over the "lower" (r < c) and
"upper" (r > c) neighbor sets:

    contrib_side(c) = [cnt>0 and poscnt==1] * (log(sum_r m exp(out_r)) - poslogit) / cnt

All per-column quantities are sums of the form sum_r adj[r,c] * w[r] for
w in {1, pos, pos*out, exp(out)} -> tensor-engine matvecs with a triangular
split. Only the DISTINCT columns referenced by idx_node (~3.2k of 8192) are
shipped; duplicates are weighted during the O(K) host combine. The adjacency
is binary by construction, so its gathered columns are shipped as fp8 (0/1
exact) -> 1 byte/element on the wire, 8x less HBM traffic than the full
int32 matrix, and row-tile PAIRS are contracted in single matmuls via the
fp8 DoubleRow perf mode.

Sharding: core d owns absolute columns [1024d, 1024d+1024). Its distinct
columns are grouped by crossing row-tile (c//128) into 8 groups of W=56
padded slots -> a [8192 x 448] fp8 slab (group-overflow columns, rare, are
computed on host). Rows are rotated by 1024d so each group's crossing tile
is local row-tile g in 0..7 -> one NEFF serves all cores. The slab is stored
tile-major in DRAM in PROCESSING order (full tiles first, crossing tiles
mid-stream) and loaded in a few large chunks; psum[16, 448] accumulates
{L,U} x {ones, pos, pl_hi, pl_lo, e_hi, e_lo} (+4 pad rows). Inside a
crossing pair, the two half-masked zones are folded into two DoubleRow
matmuls over the 112-wide region using host-packed per-subtile step masks.
"""

import numpy as np
import ml_dtypes

N = 8192
K = 4096
NCORES = 8
CRANGE = N // NCORES      # 1024 absolute columns owned per core
P = 128                   # partition / tile edge
NT = N // P               # 64 row tiles
NPAIR = NT // 2           # 32 row-tile pairs
TPC = CRANGE // P         # 8 crossing (diag) tiles per core
W = 56                    # padded column slots per crossing tile
SLAB = TPC * W            # 448 slab columns per core
RW = 2 * W                # 112-wide crossing-pair region
NW = 6                    # weights per side
M = 2 * NW                # 12 psum stat rows (L half = 0:6, U half = 6:12)
MP = 16                   # padded weight width (DoubleRow needs 16B steps)

BF16 = ml_dtypes.bfloat16
FP8 = ml_dtypes.float8_e4m3

# processing order of local row tiles: full tiles first so the head of the
# stream needs no masks, crossing tiles (0..7, extra work) mid-stream
ORDER = list(range(TPC, 52)) + list(range(0, TPC)) + list(range(52, NT))
CROSS_PAIRS = [22, 23, 24, 25]  # pair indices holding local tiles 0..7
# DMA chunk sizes in PAIRS: small head/tail pieces for pipeline latency,
# large mid chunks to keep the Sync engine's ~0.65us/trigger off the
# critical path (each trigger costs ~0.65us of serial Sync time)
CHUNKS = [1, 1, 2, 12, 12, 2, 1, 1]
assert sum(CHUNKS) == NPAIR

# wmat slot layout (each slot [P, 2, MP]): pair p2 in 0..31 -> packed
# [w(ORDER[2p2]) | w(ORDER[2p2+1])] (U-pack for crossing pairs); 32..35 ->
# L-packs of crossing pairs; 36..39 -> primary region packs [WL_j|WU_j+1];
# 40..43 -> secondary region packs [WU_j|WL_j+1]
NSLOT = NPAIR + 4 + 8

_BASS_CACHE = {}


def _build_bass():
    import concourse.tile as tile
    import concourse.mybir as mybir
    from concourse import bacc

    DR = mybir.MatmulPerfMode.DoubleRow

    # Bacc (not raw Bass): its compile() runs generate_event_semaphores,
    # which splits multi-sem waits — TRN2 instructions hold at most one.
    nc = bacc.Bacc("TRN2")
    adj = nc.dram_tensor("adj", [P, NT, SLAB], mybir.dt.float8e4, kind="ExternalInput")
    whead = nc.dram_tensor("whead", [P, 4, MP], mybir.dt.float8e4, kind="ExternalInput")
    wmat = nc.dram_tensor(
        "wmat", [P, 2 * NSLOT, MP], mybir.dt.float8e4, kind="ExternalInput"
    )
    masks = nc.dram_tensor(
        "masks", [P, 16, RW], mybir.dt.float8e4, kind="ExternalInput"
    )
    stats = nc.dram_tensor("stats", [M, SLAB], mybir.dt.float32, kind="ExternalOutput")

    with tile.TileContext(nc) as tc:
        with (
            tc.tile_pool(name="singles", bufs=1) as singles,
            tc.tile_pool(name="diag", bufs=8) as diag_pool,
            tc.tile_pool(name="psum", bufs=1, space="PSUM") as psum_pool,
        ):
            # head weights (pairs 0 and 1 only, 4KB) first so the first
            # matmul isn't gated on the full weight table's transfer
            whead_sb = singles.tile([P, 4, MP], mybir.dt.float8e4)
            nc.sync.dma_start(out=whead_sb, in_=whead[:, :, :])

            # all chunks are SBUF-resident (28KB/partition total) with no
            # pool reuse -> DMA triggers carry no reuse waits at all
            chunk_tiles = []
            pos = 0
            for i, sz in enumerate(CHUNKS):
                t = singles.tile(
                    [P, 2 * sz, SLAB], mybir.dt.float8e4,
                    tag=f"chunk{i}", name=f"chunk{i}",
                )
                chunk_tiles.append((t, pos, sz))
                nc.sync.dma_start(out=t, in_=adj[:, 2 * pos : 2 * (pos + sz), :])
                pos += sz
                if i == 0:
                    wsb = singles.tile([P, 2 * NSLOT, MP], mybir.dt.float8e4)
                    nc.sync.dma_start(out=wsb, in_=wmat[:, :, :])
                if i == 2:
                    msb_raw = singles.tile([P, 16, RW], mybir.dt.float8e4)
                    nc.sync.dma_start(out=msb_raw, in_=masks[:, :, :])

            # Re-produce the masks on DVE: the DVE TensorTensor ISA struct has
            # room for a single sync-wait, so the region-mask multiplies must
            # only ever depend on DVE-produced operands (one self-sem wait).
            msb = singles.tile([P, 16, RW], mybir.dt.float8e4)
            nc.vector.tensor_copy(msb, msb_raw)

            acc = psum_pool.tile([MP, SLAB], mybir.dt.float32, tag="acc", name="acc")

            def wpair(slot):  # [P, 2, MP] DoubleRow stationary pack
                if slot < 2:
                    return whead_sb[:, 2 * slot : 2 * slot + 2, :]
                return wsb[:, 2 * slot : 2 * slot + 2, :]

            # start=True zeroes the ENTIRE psum bank a matmul touches; SLAB
            # (448) fits one 512-col bank, so only the first matmul starts.
            state = {"started": False}

            def mm_dr(slot, rhs3, a, b, stop=False):
                # rhs3 spans psum columns [a, b); slices stay single-step APs
                if a >= b:
                    return
                nc.tensor.matmul(
                    acc[:, a:b], wpair(slot), rhs3,
                    start=not state["started"], stop=stop,
                    perf_mode=DR, skip_group_check=True,
                )
                state["started"] = True

            for t, pos0, sz in chunk_tiles:
                for k in range(sz):
                    p2 = pos0 + k  # pair index
                    last = p2 == NPAIR - 1
                    if last:
                        # fine-grained tail: shortest latency from data
                        # arrival to final matmul (bounds 16-aligned)
                        bounds = [0, 112, 224, 336, SLAB]
                        for s, e in zip(bounds[:-1], bounds[1:]):
                            mm_dr(p2, t[:, 2 * k : 2 * k + 2, s:e], s, e,
                                  stop=(e == SLAB))
                        continue
                    if p2 in CROSS_PAIRS:
                        q = p2 - CROSS_PAIRS[0]
                        j = 2 * q  # local crossing tiles j, j+1
                        zA, zC = j * W, (j + 2) * W
                        # zone A [0, zA): both tiles U -> U-pack DoubleRow
                        mm_dr(p2, t[:, 2 * k : 2 * k + 2, 0:zA], 0, zA)
                        # region [zA, zC): two DoubleRows with host-packed
                        # per-subtile step masks (see _build_inputs)
                        prim = diag_pool.tile([P, 2, RW], mybir.dt.float8e4)
                        nc.vector.tensor_mul(
                            prim, t[:, 2 * k : 2 * k + 2, zA:zC],
                            msb[:, 4 * q : 4 * q + 2, :],
                        )
                        sec = diag_pool.tile([P, 2, RW], mybir.dt.float8e4)
                        nc.vector.tensor_mul(
                            sec, t[:, 2 * k : 2 * k + 2, zA:zC],
                            msb[:, 4 * q + 2 : 4 * q + 4, :],
                        )
                        mm_dr(36 + q, prim, zA, zC)
                        mm_dr(40 + q, sec, zA, zC)
                        # zone D [zC, 448): both tiles L -> L-pack DoubleRow
                        mm_dr(NPAIR + q, t[:, 2 * k : 2 * k + 2, zC:SLAB],
                              zC, SLAB)
                    else:
                        mm_dr(p2, t[:, 2 * k : 2 * k + 2, :], 0, SLAB)

            # copy-out tail split across ACT and DVE so the halves run in
            # parallel right after the final matmul
            out_sb = singles.tile([M, SLAB], mybir.dt.float32)
            half = SLAB // 2
            nc.scalar.copy(out_sb[:, 0:half], acc[0:M, 0:half])
            nc.vector.tensor_copy(out_sb[:, half:], acc[0:M, half:])
            nc.sync.dma_start(out=stats[:, :], in_=out_sb[:, :])

    nc.compile()
    return nc


def _split_fp8(v):
    hi = v.astype(FP8)
    lo = (v - hi.astype(np.float64)).astype(FP8)
    return hi, lo


def _host_weights(outputs, targets):
    """Per-row weight table Wside [N, 6] fp8 (hi/lo split pairs)."""
    out = np.asarray(outputs, np.float64).reshape(-1)
    pos = (np.asarray(targets).reshape(-1) != 0).astype(np.float64)
    pl_hi, pl_lo = _split_fp8(pos * out)
    e_hi, e_lo = _split_fp8(np.exp(out))
    return np.stack(
        [np.ones(N, FP8), pos.astype(FP8), pl_hi, pl_lo, e_hi, e_lo], axis=1
    ).astype(FP8)  # [N, 6]


def _tile_weights(wside, core):
    """Per local tile j: (wl[128, MP], wu[128, MP]) fp8, zero-padded.

    wl has the L stats in rows 0:6, wu the U stats in rows 6:12, matching
    the psum layout; for non-crossing tiles only the relevant one is used.
    """
    wl = np.zeros((NT, P, MP), FP8)
    wu = np.zeros((NT, P, MP), FP8)
    for j in range(NT):
        t = (TPC * core + j) % NT
        rows = wside[t * P : (t + 1) * P, :]  # [128, 6]
        wl[j, :, 0:NW] = rows
        wu[j, :, NW:M] = rows
    return wl, wu


def _build_wmat(wside, core):
    """Slot-packed stationary weights [P, 2*NSLOT, MP] fp8 (see layout)."""
    wl, wu = _tile_weights(wside, core)
    w = np.zeros((P, 2 * NSLOT, MP), FP8)

    def tile_w(j):
        # routing for a full (non-crossing) tile: U if its absolute tile
        # index is above the slab's column range, else L (wrapped rows)
        return wu[j] if j < NT - TPC * core else wl[j]

    for p2 in range(NPAIR):
        j0, j1 = ORDER[2 * p2], ORDER[2 * p2 + 1]
        if p2 in CROSS_PAIRS:
            w[:, 2 * p2] = wu[j0]      # U-pack (zone A)
            w[:, 2 * p2 + 1] = wu[j1]
        else:
            w[:, 2 * p2] = tile_w(j0)
            w[:, 2 * p2 + 1] = tile_w(j1)
    for q in range(4):
        j = 2 * q
        w[:, 2 * (NPAIR + q)] = wl[j]          # L-pack (zone D)
        w[:, 2 * (NPAIR + q) + 1] = wl[j + 1]
        w[:, 2 * (36 + q)] = wl[j]             # primary region pack
        w[:, 2 * (36 + q) + 1] = wu[j + 1]
        w[:, 2 * (40 + q)] = wu[j]             # secondary region pack
        w[:, 2 * (40 + q) + 1] = wl[j + 1]
    return np.ascontiguousarray(w)


def _plan_columns(idx_node):
    """Distinct query columns -> per-core padded slot plan + host overflow."""
    idx = np.asarray(idx_node).reshape(-1).astype(np.int64)
    dist, mult = np.unique(idx, return_counts=True)
    plan = [[[] for _ in range(TPC)] for _ in range(NCORES)]
    overflow = []
    for c, m in zip(dist, mult):
        d, g = int(c) // CRANGE, (int(c) % CRANGE) // P
        if len(plan[d][g]) < W:
            plan[d][g].append((int(c), int(m)))
        else:
            overflow.append((int(c), int(m)))
    return plan, overflow


def _build_inputs(node_adj, wside, plan):
    """Per-core in_maps: tile-major rotated fp8 slab, weights, region masks."""
    node_adj = np.asarray(node_adj)
    in_maps = []
    ri = np.arange(P)[:, None]  # [128,1]
    for d in range(NCORES):
        slab = np.zeros((N, SLAB), FP8)
        maskL = np.zeros((P, SLAB), np.float32)
        maskU = np.zeros((P, SLAB), np.float32)
        c0 = CRANGE * d
        for g in range(TPC):
            entries = plan[d][g]
            if not entries:
                continue
            cols = np.array([c for c, _ in entries], np.int64)
            sl = slice(g * W, g * W + len(cols))
            gathered = (node_adj[:, cols] != 0).astype(FP8)
            # rotate rows: local row rho = (abs_row - CRANGE*d) mod N
            slab[:, sl] = np.concatenate([gathered[c0:], gathered[:c0]], axis=0)
            # strict step masks vs the crossing tile's absolute row base;
            # strictness zeroes the diagonal on both sides
            base = c0 + g * P
            maskL[:, sl] = (base + ri < cols[None, :])
            maskU[:, sl] = (base + ri > cols[None, :])
        # region mask packs [P, 16, RW]: for crossing pair q (tiles j=2q,
        # j+1), region = slab cols [jW, (j+2)W) = zones B|C:
        #   primary sub0 (tile j):   [maskL_B | ones_C]
        #   primary sub1 (tile j+1): [ones_B  | maskU_C]
        #   secondary sub0:          [maskU_B | zeros_C]
        #   secondary sub1:          [zeros_B | maskL_C]
        mp = np.zeros((P, 16, RW), FP8)
        for q in range(4):
            j = 2 * q
            zB = slice(j * W, (j + 1) * W)
            zC = slice((j + 1) * W, (j + 2) * W)
            mp[:, 4 * q + 0, 0:W] = maskL[:, zB].astype(FP8)
            mp[:, 4 * q + 0, W:RW] = 1.0
            mp[:, 4 * q + 1, 0:W] = 1.0
            mp[:, 4 * q + 1, W:RW] = maskU[:, zC].astype(FP8)
            mp[:, 4 * q + 2, 0:W] = maskU[:, zB].astype(FP8)
            mp[:, 4 * q + 3, W:RW] = maskL[:, zC].astype(FP8)
        # tile-major DRAM layout in processing order
        adjT = np.zeros((P, NT, SLAB), FP8)
        for p, j in enumerate(ORDER):
            adjT[:, p, :] = slab[j * P : (j + 1) * P, :]
        wm = _build_wmat(wside, d)
        in_maps.append(
            {
                "adj": np.ascontiguousarray(adjT),
                "whead": np.ascontiguousarray(wm[:, 0:4, :]),
                "wmat": wm,
                "masks": np.ascontiguousarray(mp),
            }
        )
    return in_maps


def _side_contrib(x):
    cnt, poscnt = x[0], x[1]
    poslogit = x[2] + x[3]
    sumexp = x[4] + x[5]
    valid = (cnt > 0.5) & (np.abs(poscnt - 1.0) < 0.25)
    lse = np.log(np.where(valid, np.maximum(sumexp, 1e-300), 1.0))
    return np.where(valid, (lse - poslogit) / np.maximum(cnt, 1.0), 0.0)


def _combine(stats_list, plan, overflow, outputs, targets, node_adj):
    """stats_list: per-core [12, SLAB] f32 -> scalar loss (f64 math)."""
    total = 0.0
    for d in range(NCORES):
        x = np.asarray(stats_list[d], np.float64)
        contrib = _side_contrib(x[0:NW]) + _side_contrib(x[NW:M])
        for g in range(TPC):
            for i, (_, m) in enumerate(plan[d][g]):
                total += m * contrib[g * W + i]
    if overflow:
        out = np.asarray(outputs, np.float64).reshape(-1)
        pos = np.asarray(targets).reshape(-1) != 0
        rows = np.arange(N)
        for c, m in overflow:
            col = (np.asarray(node_adj[:, c]).reshape(-1) != 0) & (rows != c)
            for mask in (col & (rows < c), col & (rows >= c)):
                cnt = int(mask.sum())
                pcnt = int((mask & pos).sum())
                if cnt > 0 and pcnt == 1:
                    lse = np.log(np.exp(out[mask]).sum())
                    pl = out[mask & pos].sum()
                    total += m * (lse - pl) / cnt
    return np.float32(total)


def _ensure_axon_hooks_stub():
    """bass_utils imports antenv.axon_hooks when tracing is requested via
    env; the module is absent on some images. Provide a no-op stub so the
    import never crashes (hook=None -> bass_utils skips tracing)."""
    import sys
    import types

    try:
        import antenv.axon_hooks  # noqa: F401
    except ImportError:
        mod = types.ModuleType("antenv.axon_hooks")
        state = {"hook": None}
        mod.set_axon_ntff_profile_hook = lambda h: state.__setitem__("hook", h)
        mod.get_axon_ntff_profile_hook = lambda: state["hook"]
        sys.modules["antenv.axon_hooks"] = mod


def _device_stats(in_maps):
    _ensure_axon_hooks_stub()
    from concourse.bass_utils import run_bass_kernel_spmd

    if "nc" not in _BASS_CACHE:
        _BASS_CACHE["nc"] = _build_bass()
    last_exc = None
    for attempt in range(4):
        try:
            res = run_bass_kernel_spmd(
                _BASS_CACHE["nc"], in_maps, core_ids=list(range(NCORES))
            )
            return [r["stats"] for r in res.results]
        except Exception as e:  # transient NRT/accelerator hiccups
            last_exc = e
            try:
                # a fresh PJRT client usually recovers a transiently
                # "unrecoverable" accelerator; mirrors a process restart
                import jax
                import jax.extend.backend as _jeb

                jax.clear_caches()
                _jeb.clear_backends()
            except Exception:
                pass
            import time

            time.sleep(2.0 * (attempt + 1))
    raise last_exc


def _sim_stats(in_maps):
    """Numpy emulation of the device kernel (same inputs incl. fp8
    quantization and wmat slot packing), for logic + precision validation."""
    outs = []
    for im in in_maps:
        adjT = im["adj"].astype(np.float32)           # [P, NT, SLAB]
        w = im["wmat"].astype(np.float32)             # [P, 2*NSLOT, MP]
        mp = im["masks"].astype(np.float32)           # [P, 16, RW]
        acc = np.zeros((MP, SLAB), np.float32)
        for p2 in range(NPAIR):
            a0, a1 = adjT[:, 2 * p2, :], adjT[:, 2 * p2 + 1, :]
            if p2 in CROSS_PAIRS:
                q = p2 - CROSS_PAIRS[0]
                j = 2 * q
                zA, zC = j * W, (j + 2) * W
                acc[:, :zA] += w[:, 2 * p2].T @ a0[:, :zA]
                acc[:, :zA] += w[:, 2 * p2 + 1].T @ a1[:, :zA]
                reg0, reg1 = a0[:, zA:zC], a1[:, zA:zC]
                acc[:, zA:zC] += w[:, 2 * (36 + q)].T @ (reg0 * mp[:, 4 * q + 0])
                acc[:, zA:zC] += w[:, 2 * (36 + q) + 1].T @ (reg1 * mp[:, 4 * q + 1])
                acc[:, zA:zC] += w[:, 2 * (40 + q)].T @ (reg0 * mp[:, 4 * q + 2])
                acc[:, zA:zC] += w[:, 2 * (40 + q) + 1].T @ (reg1 * mp[:, 4 * q + 3])
                acc[:, zC:] += w[:, 2 * (NPAIR + q)].T @ a0[:, zC:]
                acc[:, zC:] += w[:, 2 * (NPAIR + q) + 1].T @ a1[:, zC:]
            else:
                acc += w[:, 2 * p2].T @ a0
                acc += w[:, 2 * p2 + 1].T @ a1
        outs.append(acc[0:M])
    return outs


def prepare(outputs, targets, node_adj, idx_node):
    wside = _host_weights(outputs, targets)
    plan, overflow = _plan_columns(idx_node)
    in_maps = _build_inputs(node_adj, wside, plan)
    return in_maps, plan, overflow


def kernel(outputs, targets, node_adj, idx_node, _simulate=False):
    in_maps, plan, overflow = prepare(outputs, targets, node_adj, idx_node)
    stats = _sim_stats(in_maps) if _simulate else _device_stats(in_maps)
    return _combine(stats, plan, overflow, outputs, targets, node_adj)


# revision 10
# speedup vs baseline: 1.0414x; 1.0414x over previous
"""Trainium2 Bass kernel for nn_CELoss_4896262717859.

Computes, for each query column c = idx_node[k] of a sparse adjacency matrix
(diagonal zeroed), a cross-entropy-style loss over the "lower" (r < c) and
"upper" (r > c) neighbor sets:

    contrib_side(c) = [cnt>0 and poscnt==1] * (log(sum_r m exp(out_r)) - poslogit) / cnt

All per-column quantities are sums of the form sum_r adj[r,c] * w[r] for
w in {1, pos, pos*out, exp(out)} -> tensor-engine matvecs with a triangular
split. Only the DISTINCT columns referenced by idx_node (~3.2k of 8192) are
shipped; duplicates are weighted during the O(K) host combine. The adjacency
is binary by construction, so its gathered columns are shipped as fp8 (0/1
exact) -> 1 byte/element on the wire, 8x less HBM traffic than the full
int32 matrix, and row-tile PAIRS are contracted in single matmuls via the
fp8 DoubleRow perf mode.

Sharding: core d owns absolute columns [1024d, 1024d+1024). Its distinct
columns are grouped by crossing row-tile (c//128) into 8 groups of W=56
padded slots -> a [8192 x 448] fp8 slab (group-overflow columns, rare, are
computed on host). Rows are rotated by 1024d so each group's crossing tile
is local row-tile g in 0..7 -> one NEFF serves all cores. The slab is stored
tile-major in DRAM in PROCESSING order (full tiles first, crossing tiles
mid-stream) and loaded in a few large chunks; psum[16, 448] accumulates
{L,U} x {ones, pos, pl_hi, pl_lo, e_hi, e_lo} (+4 pad rows). Inside a
crossing pair, the two half-masked zones are folded into two DoubleRow
matmuls over the 112-wide region using host-packed per-subtile step masks.
"""

import numpy as np
import ml_dtypes

N = 8192
K = 4096
NCORES = 8
CRANGE = N // NCORES      # 1024 absolute columns owned per core
P = 128                   # partition / tile edge
NT = N // P               # 64 row tiles
NPAIR = NT // 2           # 32 row-tile pairs
TPC = CRANGE // P         # 8 crossing (diag) tiles per core
W = 56                    # padded column slots per crossing tile
SLAB = TPC * W            # 448 slab columns per core
RW = 2 * W                # 112-wide crossing-pair region
NW = 6                    # weights per side
M = 2 * NW                # 12 psum stat rows (L half = 0:6, U half = 6:12)
MP = 16                   # padded weight width (DoubleRow needs 16B steps)

BF16 = ml_dtypes.bfloat16
FP8 = ml_dtypes.float8_e4m3

# processing order of local row tiles: full tiles first so the head of the
# stream needs no masks, crossing tiles (0..7, extra work) mid-stream
ORDER = list(range(TPC, 52)) + list(range(0, TPC)) + list(range(52, NT))
CROSS_PAIRS = [22, 23, 24, 25]  # pair indices holding local tiles 0..7
# DMA chunk sizes in PAIRS: small head/tail pieces for pipeline latency,
# large mid chunks to keep the Sync engine's ~0.65us/trigger off the
# critical path (each trigger costs ~0.65us of serial Sync time)
CHUNKS = [1, 1, 2, 4, 8, 8, 4, 2, 1, 1]
assert sum(CHUNKS) == NPAIR

# wmat slot layout (each slot [P, 2, MP]): pair p2 in 0..31 -> packed
# [w(ORDER[2p2]) | w(ORDER[2p2+1])] (U-pack for crossing pairs); 32..35 ->
# L-packs of crossing pairs; 36..39 -> primary region packs [WL_j|WU_j+1];
# 40..43 -> secondary region packs [WU_j|WL_j+1]
NSLOT = NPAIR + 4 + 8

_BASS_CACHE = {}


def _build_bass():
    import concourse.tile as tile
    import concourse.mybir as mybir
    from concourse import bacc

    DR = mybir.MatmulPerfMode.DoubleRow

    # Bacc (not raw Bass): its compile() runs generate_event_semaphores,
    # which splits multi-sem waits — TRN2 instructions hold at most one.
    nc = bacc.Bacc("TRN2")
    adj = nc.dram_tensor("adj", [P, NT, SLAB], mybir.dt.float8e4, kind="ExternalInput")
    whead = nc.dram_tensor("whead", [P, 4, MP], mybir.dt.float8e4, kind="ExternalInput")
    wmat = nc.dram_tensor(
        "wmat", [P, 2 * NSLOT, MP], mybir.dt.float8e4, kind="ExternalInput"
    )
    masks = nc.dram_tensor(
        "masks", [P, 16, RW], mybir.dt.float8e4, kind="ExternalInput"
    )
    stats = nc.dram_tensor("stats", [M, SLAB], mybir.dt.float32, kind="ExternalOutput")

    with tile.TileContext(nc) as tc:
        with (
            tc.tile_pool(name="singles", bufs=1) as singles,
            tc.tile_pool(name="diag", bufs=8) as diag_pool,
            tc.tile_pool(name="psum", bufs=1, space="PSUM") as psum_pool,
        ):
            # all chunks are SBUF-resident (28KB/partition total) with no
            # pool reuse -> DMA triggers carry no reuse waits at all.
            # Order: first adjacency pair, then the tiny head weights (4KB,
            # pairs 0-1 only) so the first matmul is gated on neither the
            # full weight table nor a crowd of concurrent transfers.
            chunk_tiles = []
            pos = 0
            for i, sz in enumerate(CHUNKS):
                t = singles.tile(
                    [P, 2 * sz, SLAB], mybir.dt.float8e4,
                    tag=f"chunk{i}", name=f"chunk{i}",
                )
                chunk_tiles.append((t, pos, sz))
                nc.sync.dma_start(out=t, in_=adj[:, 2 * pos : 2 * (pos + sz), :])
                pos += sz
                if i == 0:
                    whead_sb = singles.tile([P, 4, MP], mybir.dt.float8e4)
                    nc.sync.dma_start(out=whead_sb, in_=whead[:, :, :])
                if i == 1:
                    wsb = singles.tile([P, 2 * NSLOT, MP], mybir.dt.float8e4)
                    nc.sync.dma_start(out=wsb, in_=wmat[:, :, :])
                if i == 2:
                    msb_raw = singles.tile([P, 16, RW], mybir.dt.float8e4)
                    nc.sync.dma_start(out=msb_raw, in_=masks[:, :, :])

            # Re-produce the masks on DVE: the DVE TensorTensor ISA struct has
            # room for a single sync-wait, so the region-mask multiplies must
            # only ever depend on DVE-produced operands (one self-sem wait).
            msb = singles.tile([P, 16, RW], mybir.dt.float8e4)
            nc.vector.tensor_copy(msb, msb_raw)

            acc = psum_pool.tile([MP, SLAB], mybir.dt.float32, tag="acc", name="acc")

            def wpair(slot):  # [P, 2, MP] DoubleRow stationary pack
                if slot < 2:
                    return whead_sb[:, 2 * slot : 2 * slot + 2, :]
                return wsb[:, 2 * slot : 2 * slot + 2, :]

            # start=True zeroes the ENTIRE psum bank a matmul touches; SLAB
            # (448) fits one 512-col bank, so only the first matmul starts.
            state = {"started": False}

            def mm_dr(slot, rhs3, a, b, stop=False):
                # rhs3 spans psum columns [a, b); slices stay single-step APs
                if a >= b:
                    return
                nc.tensor.matmul(
                    acc[:, a:b], wpair(slot), rhs3,
                    start=not state["started"], stop=stop,
                    perf_mode=DR, skip_group_check=True,
                )
                state["started"] = True

            for t, pos0, sz in chunk_tiles:
                for k in range(sz):
                    p2 = pos0 + k  # pair index
                    last = p2 == NPAIR - 1
                    if last:
                        # fine-grained tail: shortest latency from data
                        # arrival to final matmul (bounds 16-aligned)
                        bounds = [0, 112, 224, 336, SLAB]
                        for s, e in zip(bounds[:-1], bounds[1:]):
                            mm_dr(p2, t[:, 2 * k : 2 * k + 2, s:e], s, e,
                                  stop=(e == SLAB))
                        continue
                    if p2 in CROSS_PAIRS:
                        q = p2 - CROSS_PAIRS[0]
                        j = 2 * q  # local crossing tiles j, j+1
                        zA, zC = j * W, (j + 2) * W
                        # zone A [0, zA): both tiles U -> U-pack DoubleRow
                        mm_dr(p2, t[:, 2 * k : 2 * k + 2, 0:zA], 0, zA)
                        # region [zA, zC): two DoubleRows with host-packed
                        # per-subtile step masks (see _build_inputs)
                        prim = diag_pool.tile([P, 2, RW], mybir.dt.float8e4)
                        nc.vector.tensor_mul(
                            prim, t[:, 2 * k : 2 * k + 2, zA:zC],
                            msb[:, 4 * q : 4 * q + 2, :],
                        )
                        sec = diag_pool.tile([P, 2, RW], mybir.dt.float8e4)
                        nc.vector.tensor_mul(
                            sec, t[:, 2 * k : 2 * k + 2, zA:zC],
                            msb[:, 4 * q + 2 : 4 * q + 4, :],
                        )
                        mm_dr(36 + q, prim, zA, zC)
                        mm_dr(40 + q, sec, zA, zC)
                        # zone D [zC, 448): both tiles L -> L-pack DoubleRow
                        mm_dr(NPAIR + q, t[:, 2 * k : 2 * k + 2, zC:SLAB],
                              zC, SLAB)
                    else:
                        mm_dr(p2, t[:, 2 * k : 2 * k + 2, :], 0, SLAB)

            # copy-out tail split across ACT and DVE so the halves run in
            # parallel right after the final matmul
            out_sb = singles.tile([M, SLAB], mybir.dt.float32)
            half = SLAB // 2
            nc.scalar.copy(out_sb[:, 0:half], acc[0:M, 0:half])
            nc.vector.tensor_copy(out_sb[:, half:], acc[0:M, half:])
            nc.sync.dma_start(out=stats[:, :], in_=out_sb[:, :])

    nc.compile()
    return nc


def _split_fp8(v):
    hi = v.astype(FP8)
    lo = (v - hi.astype(np.float64)).astype(FP8)
    return hi, lo


def _host_weights(outputs, targets):
    """Per-row weight table Wside [N, 6] fp8 (hi/lo split pairs)."""
    out = np.asarray(outputs, np.float64).reshape(-1)
    pos = (np.asarray(targets).reshape(-1) != 0).astype(np.float64)
    pl_hi, pl_lo = _split_fp8(pos * out)
    e_hi, e_lo = _split_fp8(np.exp(out))
    return np.stack(
        [np.ones(N, FP8), pos.astype(FP8), pl_hi, pl_lo, e_hi, e_lo], axis=1
    ).astype(FP8)  # [N, 6]


def _tile_weights(wside, core):
    """Per local tile j: (wl[128, MP], wu[128, MP]) fp8, zero-padded.

    wl has the L stats in rows 0:6, wu the U stats in rows 6:12, matching
    the psum layout; for non-crossing tiles only the relevant one is used.
    """
    wl = np.zeros((NT, P, MP), FP8)
    wu = np.zeros((NT, P, MP), FP8)
    for j in range(NT):
        t = (TPC * core + j) % NT
        rows = wside[t * P : (t + 1) * P, :]  # [128, 6]
        wl[j, :, 0:NW] = rows
        wu[j, :, NW:M] = rows
    return wl, wu


def _build_wmat(wside, core):
    """Slot-packed stationary weights [P, 2*NSLOT, MP] fp8 (see layout)."""
    wl, wu = _tile_weights(wside, core)
    w = np.zeros((P, 2 * NSLOT, MP), FP8)

    def tile_w(j):
        # routing for a full (non-crossing) tile: U if its absolute tile
        # index is above the slab's column range, else L (wrapped rows)
        return wu[j] if j < NT - TPC * core else wl[j]

    for p2 in range(NPAIR):
        j0, j1 = ORDER[2 * p2], ORDER[2 * p2 + 1]
        if p2 in CROSS_PAIRS:
            w[:, 2 * p2] = wu[j0]      # U-pack (zone A)
            w[:, 2 * p2 + 1] = wu[j1]
        else:
            w[:, 2 * p2] = tile_w(j0)
            w[:, 2 * p2 + 1] = tile_w(j1)
    for q in range(4):
        j = 2 * q
        w[:, 2 * (NPAIR + q)] = wl[j]          # L-pack (zone D)
        w[:, 2 * (NPAIR + q) + 1] = wl[j + 1]
        w[:, 2 * (36 + q)] = wl[j]             # primary region pack
        w[:, 2 * (36 + q) + 1] = wu[j + 1]
        w[:, 2 * (40 + q)] = wu[j]             # secondary region pack
        w[:, 2 * (40 + q) + 1] = wl[j + 1]
    return np.ascontiguousarray(w)


def _plan_columns(idx_node):
    """Distinct query columns -> per-core padded slot plan + host overflow."""
    idx = np.asarray(idx_node).reshape(-1).astype(np.int64)
    dist, mult = np.unique(idx, return_counts=True)
    plan = [[[] for _ in range(TPC)] for _ in range(NCORES)]
    overflow = []
    for c, m in zip(dist, mult):
        d, g = int(c) // CRANGE, (int(c) % CRANGE) // P
        if len(plan[d][g]) < W:
            plan[d][g].append((int(c), int(m)))
        else:
            overflow.append((int(c), int(m)))
    return plan, overflow


def _build_inputs(node_adj, wside, plan):
    """Per-core in_maps: tile-major rotated fp8 slab, weights, region masks."""
    node_adj = np.asarray(node_adj)
    in_maps = []
    ri = np.arange(P)[:, None]  # [128,1]
    for d in range(NCORES):
        slab = np.zeros((N, SLAB), FP8)
        maskL = np.zeros((P, SLAB), np.float32)
        maskU = np.zeros((P, SLAB), np.float32)
        c0 = CRANGE * d
        for g in range(TPC):
            entries = plan[d][g]
            if not entries:
                continue
            cols = np.array([c for c, _ in entries], np.int64)
            sl = slice(g * W, g * W + len(cols))
            gathered = (node_adj[:, cols] != 0).astype(FP8)
            # rotate rows: local row rho = (abs_row - CRANGE*d) mod N
            slab[:, sl] = np.concatenate([gathered[c0:], gathered[:c0]], axis=0)
            # strict step masks vs the crossing tile's absolute row base;
            # strictness zeroes the diagonal on both sides
            base = c0 + g * P
            maskL[:, sl] = (base + ri < cols[None, :])
            maskU[:, sl] = (base + ri > cols[None, :])
        # region mask packs [P, 16, RW]: for crossing pair q (tiles j=2q,
        # j+1), region = slab cols [jW, (j+2)W) = zones B|C:
        #   primary sub0 (tile j):   [maskL_B | ones_C]
        #   primary sub1 (tile j+1): [ones_B  | maskU_C]
        #   secondary sub0:          [maskU_B | zeros_C]
        #   secondary sub1:          [zeros_B | maskL_C]
        mp = np.zeros((P, 16, RW), FP8)
        for q in range(4):
            j = 2 * q
            zB = slice(j * W, (j + 1) * W)
            zC = slice((j + 1) * W, (j + 2) * W)
            mp[:, 4 * q + 0, 0:W] = maskL[:, zB].astype(FP8)
            mp[:, 4 * q + 0, W:RW] = 1.0
            mp[:, 4 * q + 1, 0:W] = 1.0
            mp[:, 4 * q + 1, W:RW] = maskU[:, zC].astype(FP8)
            mp[:, 4 * q + 2, 0:W] = maskU[:, zB].astype(FP8)
            mp[:, 4 * q + 3, W:RW] = maskL[:, zC].astype(FP8)
        # tile-major DRAM layout in processing order
        adjT = np.zeros((P, NT, SLAB), FP8)
        for p, j in enumerate(ORDER):
            adjT[:, p, :] = slab[j * P : (j + 1) * P, :]
        wm = _build_wmat(wside, d)
        in_maps.append(
            {
                "adj": np.ascontiguousarray(adjT),
                "whead": np.ascontiguousarray(wm[:, 0:4, :]),
                "wmat": wm,
                "masks": np.ascontiguousarray(mp),
            }
        )
    return in_maps


def _side_contrib(x):
    cnt, poscnt = x[0], x[1]
    poslogit = x[2] + x[3]
    sumexp = x[4] + x[5]
    valid = (cnt > 0.5) & (np.abs(poscnt - 1.0) < 0.25)
    lse = np.log(np.where(valid, np.maximum(sumexp, 1e-300), 1.0))
    return np.where(valid, (lse - poslogit) / np.maximum(cnt, 1.0), 0.0)


def _combine(stats_list, plan, overflow, outputs, targets, node_adj):
    """stats_list: per-core [12, SLAB] f32 -> scalar loss (f64 math)."""
    total = 0.0
    for d in range(NCORES):
        x = np.asarray(stats_list[d], np.float64)
        contrib = _side_contrib(x[0:NW]) + _side_contrib(x[NW:M])
        for g in range(TPC):
            for i, (_, m) in enumerate(plan[d][g]):
                total += m * contrib[g * W + i]
    if overflow:
        out = np.asarray(outputs, np.float64).reshape(-1)
        pos = np.asarray(targets).reshape(-1) != 0
        rows = np.arange(N)
        for c, m in overflow:
            col = (np.asarray(node_adj[:, c]).reshape(-1) != 0) & (rows != c)
            for mask in (col & (rows < c), col & (rows >= c)):
                cnt = int(mask.sum())
                pcnt = int((mask & pos).sum())
                if cnt > 0 and pcnt == 1:
                    lse = np.log(np.exp(out[mask]).sum())
                    pl = out[mask & pos].sum()
                    total += m * (lse - pl) / cnt
    return np.float32(total)


def _ensure_axon_hooks_stub():
    """bass_utils imports antenv.axon_hooks when tracing is requested via
    env; the module is absent on some images. Provide a no-op stub so the
    import never crashes (hook=None -> bass_utils skips tracing)."""
    import sys
    import types

    try:
        import antenv.axon_hooks  # noqa: F401
    except ImportError:
        mod = types.ModuleType("antenv.axon_hooks")
        state = {"hook": None}
        mod.set_axon_ntff_profile_hook = lambda h: state.__setitem__("hook", h)
        mod.get_axon_ntff_profile_hook = lambda: state["hook"]
        sys.modules["antenv.axon_hooks"] = mod


def _device_stats(in_maps):
    _ensure_axon_hooks_stub()
    from concourse.bass_utils import run_bass_kernel_spmd

    if "nc" not in _BASS_CACHE:
        _BASS_CACHE["nc"] = _build_bass()
    last_exc = None
    for attempt in range(4):
        try:
            res = run_bass_kernel_spmd(
                _BASS_CACHE["nc"], in_maps, core_ids=list(range(NCORES))
            )
            return [r["stats"] for r in res.results]
        except Exception as e:  # transient NRT/accelerator hiccups
            last_exc = e
            try:
                # a fresh PJRT client usually recovers a transiently
                # "unrecoverable" accelerator; mirrors a process restart
                import jax
                import jax.extend.backend as _jeb

                jax.clear_caches()
                _jeb.clear_backends()
            except Exception:
                pass
            import time

            time.sleep(2.0 * (attempt + 1))
    raise last_exc


def _sim_stats(in_maps):
    """Numpy emulation of the device kernel (same inputs incl. fp8
    quantization and wmat slot packing), for logic + precision validation."""
    outs = []
    for im in in_maps:
        adjT = im["adj"].astype(np.float32)           # [P, NT, SLAB]
        w = im["wmat"].astype(np.float32)             # [P, 2*NSLOT, MP]
        mp = im["masks"].astype(np.float32)           # [P, 16, RW]
        acc = np.zeros((MP, SLAB), np.float32)
        for p2 in range(NPAIR):
            a0, a1 = adjT[:, 2 * p2, :], adjT[:, 2 * p2 + 1, :]
            if p2 in CROSS_PAIRS:
                q = p2 - CROSS_PAIRS[0]
                j = 2 * q
                zA, zC = j * W, (j + 2) * W
                acc[:, :zA] += w[:, 2 * p2].T @ a0[:, :zA]
                acc[:, :zA] += w[:, 2 * p2 + 1].T @ a1[:, :zA]
                reg0, reg1 = a0[:, zA:zC], a1[:, zA:zC]
                acc[:, zA:zC] += w[:, 2 * (36 + q)].T @ (reg0 * mp[:, 4 * q + 0])
                acc[:, zA:zC] += w[:, 2 * (36 + q) + 1].T @ (reg1 * mp[:, 4 * q + 1])
                acc[:, zA:zC] += w[:, 2 * (40 + q)].T @ (reg0 * mp[:, 4 * q + 2])
                acc[:, zA:zC] += w[:, 2 * (40 + q) + 1].T @ (reg1 * mp[:, 4 * q + 3])
                acc[:, zC:] += w[:, 2 * (NPAIR + q)].T @ a0[:, zC:]
                acc[:, zC:] += w[:, 2 * (NPAIR + q) + 1].T @ a1[:, zC:]
            else:
                acc += w[:, 2 * p2].T @ a0
                acc += w[:, 2 * p2 + 1].T @ a1
        outs.append(acc[0:M])
    return outs


def prepare(outputs, targets, node_adj, idx_node):
    wside = _host_weights(outputs, targets)
    plan, overflow = _plan_columns(idx_node)
    in_maps = _build_inputs(node_adj, wside, plan)
    return in_maps, plan, overflow


def kernel(outputs, targets, node_adj, idx_node, _simulate=False):
    in_maps, plan, overflow = prepare(outputs, targets, node_adj, idx_node)
    stats = _sim_stats(in_maps) if _simulate else _device_stats(in_maps)
    return _combine(stats, plan, overflow, outputs, targets, node_adj)


# revision 11
# speedup vs baseline: 1.1923x; 1.1449x over previous
"""Trainium2 Bass kernel for nn_CELoss_4896262717859.

Computes, for each query column c = idx_node[k] of a sparse adjacency matrix
(diagonal zeroed), a cross-entropy-style loss over the "lower" (r < c) and
"upper" (r > c) neighbor sets:

    contrib_side(c) = [cnt>0 and poscnt==1] * (log(sum_r m exp(out_r)) - poslogit) / cnt

All per-column quantities are sums of the form sum_r adj[r,c] * w[r] for
w in {1, pos, pos*out, exp(out)} -> tensor-engine matvecs with a triangular
split. Only the DISTINCT columns referenced by idx_node (~3.2k of 8192) are
shipped; duplicates are weighted during the O(K) host combine. The adjacency
is binary by construction, so its gathered columns are shipped as fp8 (0/1
exact) -> 1 byte/element on the wire, 8x less HBM traffic than the full
int32 matrix, and row-tile PAIRS are contracted in single matmuls via the
fp8 DoubleRow perf mode.

Sharding: core d owns absolute columns [1024d, 1024d+1024). Its distinct
columns are grouped by crossing row-tile (c//128) into 8 groups of W=56
padded slots -> a [8192 x 448] fp8 slab (group-overflow columns, rare, are
computed on host). Rows are rotated by 1024d so each group's crossing tile
is local row-tile g in 0..7 -> one NEFF serves all cores. The slab is stored
tile-major in DRAM in PROCESSING order (full tiles first, crossing tiles
mid-stream) and loaded in a few large chunks; psum[16, 448] accumulates
{L,U} x {ones, pos, pl_hi, pl_lo, e_hi, e_lo} (+4 pad rows). Inside a
crossing pair, the two half-masked zones are folded into two DoubleRow
matmuls over the 112-wide region using host-packed per-subtile step masks.
"""

import numpy as np
import ml_dtypes

N = 8192
K = 4096
NCORES = 8
CRANGE = N // NCORES      # 1024 absolute columns owned per core
P = 128                   # partition / tile edge
NT = N // P               # 64 row tiles
NPAIR = NT // 2           # 32 row-tile pairs
TPC = CRANGE // P         # 8 crossing (diag) tiles per core
W = 56                    # padded column slots per crossing tile
SLAB = TPC * W            # 448 slab columns per core
RW = 2 * W                # 112-wide crossing-pair region
NW = 6                    # weights per side
M = 2 * NW                # 12 psum stat rows (L half = 0:6, U half = 6:12)
MP = 16                   # padded weight width (DoubleRow needs 16B steps)

BF16 = ml_dtypes.bfloat16
FP8 = ml_dtypes.float8_e4m3

# processing order of local row tiles: a few full tiles first (the head
# needs no masks), then the crossing tiles (0..7, whose DVE->PE chains hide
# under mid-stream DMA), then the remaining full tiles chasing the delivery
# curve with latency-free plain DoubleRows
ORDER = list(range(TPC, 2 * TPC)) + list(range(0, TPC)) + list(range(2 * TPC, NT))
CROSS_PAIRS = [4, 5, 6, 7]  # pair indices holding local tiles 0..7
# DMA chunk sizes in PAIRS: small head/tail pieces for pipeline latency,
# large mid chunks to keep the Sync engine's ~0.65us/trigger off the
# critical path (each trigger costs ~0.65us of serial Sync time)
CHUNKS = [1, 1, 2, 4, 8, 8, 4, 2, 1, 1]
assert sum(CHUNKS) == NPAIR

# wmat slot layout (each slot [P, 2, MP]): pair p2 in 0..31 -> packed
# [w(ORDER[2p2]) | w(ORDER[2p2+1])] (U-pack for crossing pairs); 32..35 ->
# L-packs of crossing pairs; 36..39 -> primary region packs [WL_j|WU_j+1];
# 40..43 -> secondary region packs [WU_j|WL_j+1]
NSLOT = NPAIR + 4 + 8

_BASS_CACHE = {}


def _build_bass():
    import concourse.tile as tile
    import concourse.mybir as mybir
    from concourse import bacc

    DR = mybir.MatmulPerfMode.DoubleRow

    # Bacc (not raw Bass): its compile() runs generate_event_semaphores,
    # which splits multi-sem waits — TRN2 instructions hold at most one.
    nc = bacc.Bacc("TRN2")
    adj = nc.dram_tensor("adj", [P, NT, SLAB], mybir.dt.float8e4, kind="ExternalInput")
    whead = nc.dram_tensor("whead", [P, 4, MP], mybir.dt.float8e4, kind="ExternalInput")
    wmat = nc.dram_tensor(
        "wmat", [P, 2 * NSLOT, MP], mybir.dt.float8e4, kind="ExternalInput"
    )
    masks = nc.dram_tensor(
        "masks", [P, 16, RW], mybir.dt.float8e4, kind="ExternalInput"
    )
    stats = nc.dram_tensor("stats", [M, SLAB], mybir.dt.float32, kind="ExternalOutput")

    with tile.TileContext(nc) as tc:
        with (
            tc.tile_pool(name="singles", bufs=1) as singles,
            tc.tile_pool(name="diag", bufs=8) as diag_pool,
            tc.tile_pool(name="psum", bufs=1, space="PSUM") as psum_pool,
        ):
            # all chunks are SBUF-resident (28KB/partition total) with no
            # pool reuse -> DMA triggers carry no reuse waits at all.
            # Order: first adjacency pair, then the tiny head weights (4KB,
            # pairs 0-1 only) so the first matmul is gated on neither the
            # full weight table nor a crowd of concurrent transfers.
            chunk_tiles = []
            pos = 0
            for i, sz in enumerate(CHUNKS):
                t = singles.tile(
                    [P, 2 * sz, SLAB], mybir.dt.float8e4,
                    tag=f"chunk{i}", name=f"chunk{i}",
                )
                chunk_tiles.append((t, pos, sz))
                nc.sync.dma_start(out=t, in_=adj[:, 2 * pos : 2 * (pos + sz), :])
                pos += sz
                if i == 0:
                    whead_sb = singles.tile([P, 4, MP], mybir.dt.float8e4)
                    nc.sync.dma_start(out=whead_sb, in_=whead[:, :, :])
                if i == 1:
                    wsb = singles.tile([P, 2 * NSLOT, MP], mybir.dt.float8e4)
                    nc.sync.dma_start(out=wsb, in_=wmat[:, :, :])
                if i == 2:
                    msb_raw = singles.tile([P, 16, RW], mybir.dt.float8e4)
                    nc.sync.dma_start(out=msb_raw, in_=masks[:, :, :])

            # Re-produce the masks on DVE: the DVE TensorTensor ISA struct has
            # room for a single sync-wait, so the region-mask multiplies must
            # only ever depend on DVE-produced operands (one self-sem wait).
            msb = singles.tile([P, 16, RW], mybir.dt.float8e4)
            nc.vector.tensor_copy(msb, msb_raw)

            acc = psum_pool.tile([MP, SLAB], mybir.dt.float32, tag="acc", name="acc")

            def wpair(slot):  # [P, 2, MP] DoubleRow stationary pack
                if slot < 2:
                    return whead_sb[:, 2 * slot : 2 * slot + 2, :]
                return wsb[:, 2 * slot : 2 * slot + 2, :]

            # start=True zeroes the ENTIRE psum bank a matmul touches; SLAB
            # (448) fits one 512-col bank, so only the first matmul starts.
            state = {"started": False}

            def mm_dr(slot, rhs3, a, b, stop=False):
                # rhs3 spans psum columns [a, b); slices stay single-step APs
                if a >= b:
                    return
                nc.tensor.matmul(
                    acc[:, a:b], wpair(slot), rhs3,
                    start=not state["started"], stop=stop,
                    perf_mode=DR, skip_group_check=True,
                )
                state["started"] = True

            for t, pos0, sz in chunk_tiles:
                for k in range(sz):
                    p2 = pos0 + k  # pair index
                    last = p2 == NPAIR - 1
                    if last:
                        # fine-grained tail: shortest latency from data
                        # arrival to final matmul (bounds 16-aligned)
                        bounds = [0, 112, 224, 336, SLAB]
                        for s, e in zip(bounds[:-1], bounds[1:]):
                            mm_dr(p2, t[:, 2 * k : 2 * k + 2, s:e], s, e,
                                  stop=(e == SLAB))
                        continue
                    if p2 in CROSS_PAIRS:
                        q = p2 - CROSS_PAIRS[0]
                        j = 2 * q  # local crossing tiles j, j+1
                        zA, zC = j * W, (j + 2) * W
                        # zone A [0, zA): both tiles U -> U-pack DoubleRow
                        mm_dr(p2, t[:, 2 * k : 2 * k + 2, 0:zA], 0, zA)
                        # region [zA, zC): two DoubleRows with host-packed
                        # per-subtile step masks (see _build_inputs)
                        prim = diag_pool.tile([P, 2, RW], mybir.dt.float8e4)
                        nc.vector.tensor_mul(
                            prim, t[:, 2 * k : 2 * k + 2, zA:zC],
                            msb[:, 4 * q : 4 * q + 2, :],
                        )
                        sec = diag_pool.tile([P, 2, RW], mybir.dt.float8e4)
                        nc.vector.tensor_mul(
                            sec, t[:, 2 * k : 2 * k + 2, zA:zC],
                            msb[:, 4 * q + 2 : 4 * q + 4, :],
                        )
                        mm_dr(36 + q, prim, zA, zC)
                        mm_dr(40 + q, sec, zA, zC)
                        # zone D [zC, 448): both tiles L -> L-pack DoubleRow
                        mm_dr(NPAIR + q, t[:, 2 * k : 2 * k + 2, zC:SLAB],
                              zC, SLAB)
                    else:
                        mm_dr(p2, t[:, 2 * k : 2 * k + 2, :], 0, SLAB)

            # copy-out tail split across ACT and DVE so the halves run in
            # parallel right after the final matmul
            out_sb = singles.tile([M, SLAB], mybir.dt.float32)
            half = SLAB // 2
            nc.scalar.copy(out_sb[:, 0:half], acc[0:M, 0:half])
            nc.vector.tensor_copy(out_sb[:, half:], acc[0:M, half:])
            nc.sync.dma_start(out=stats[:, :], in_=out_sb[:, :])

    nc.compile()
    return nc


def _split_fp8(v):
    hi = v.astype(FP8)
    lo = (v - hi.astype(np.float64)).astype(FP8)
    return hi, lo


def _host_weights(outputs, targets):
    """Per-row weight table Wside [N, 6] fp8 (hi/lo split pairs)."""
    out = np.asarray(outputs, np.float64).reshape(-1)
    pos = (np.asarray(targets).reshape(-1) != 0).astype(np.float64)
    pl_hi, pl_lo = _split_fp8(pos * out)
    e_hi, e_lo = _split_fp8(np.exp(out))
    return np.stack(
        [np.ones(N, FP8), pos.astype(FP8), pl_hi, pl_lo, e_hi, e_lo], axis=1
    ).astype(FP8)  # [N, 6]


def _tile_weights(wside, core):
    """Per local tile j: (wl[128, MP], wu[128, MP]) fp8, zero-padded.

    wl has the L stats in rows 0:6, wu the U stats in rows 6:12, matching
    the psum layout; for non-crossing tiles only the relevant one is used.
    """
    wl = np.zeros((NT, P, MP), FP8)
    wu = np.zeros((NT, P, MP), FP8)
    for j in range(NT):
        t = (TPC * core + j) % NT
        rows = wside[t * P : (t + 1) * P, :]  # [128, 6]
        wl[j, :, 0:NW] = rows
        wu[j, :, NW:M] = rows
    return wl, wu


def _build_wmat(wside, core):
    """Slot-packed stationary weights [P, 2*NSLOT, MP] fp8 (see layout)."""
    wl, wu = _tile_weights(wside, core)
    w = np.zeros((P, 2 * NSLOT, MP), FP8)

    def tile_w(j):
        # routing for a full (non-crossing) tile: U if its absolute tile
        # index is above the slab's column range, else L (wrapped rows)
        return wu[j] if j < NT - TPC * core else wl[j]

    for p2 in range(NPAIR):
        j0, j1 = ORDER[2 * p2], ORDER[2 * p2 + 1]
        if p2 in CROSS_PAIRS:
            w[:, 2 * p2] = wu[j0]      # U-pack (zone A)
            w[:, 2 * p2 + 1] = wu[j1]
        else:
            w[:, 2 * p2] = tile_w(j0)
            w[:, 2 * p2 + 1] = tile_w(j1)
    for q in range(4):
        j = 2 * q
        w[:, 2 * (NPAIR + q)] = wl[j]          # L-pack (zone D)
        w[:, 2 * (NPAIR + q) + 1] = wl[j + 1]
        w[:, 2 * (36 + q)] = wl[j]             # primary region pack
        w[:, 2 * (36 + q) + 1] = wu[j + 1]
        w[:, 2 * (40 + q)] = wu[j]             # secondary region pack
        w[:, 2 * (40 + q) + 1] = wl[j + 1]
    return np.ascontiguousarray(w)


def _plan_columns(idx_node):
    """Distinct query columns -> per-core padded slot plan + host overflow."""
    idx = np.asarray(idx_node).reshape(-1).astype(np.int64)
    dist, mult = np.unique(idx, return_counts=True)
    plan = [[[] for _ in range(TPC)] for _ in range(NCORES)]
    overflow = []
    for c, m in zip(dist, mult):
        d, g = int(c) // CRANGE, (int(c) % CRANGE) // P
        if len(plan[d][g]) < W:
            plan[d][g].append((int(c), int(m)))
        else:
            overflow.append((int(c), int(m)))
    return plan, overflow


def _build_inputs(node_adj, wside, plan):
    """Per-core in_maps: tile-major rotated fp8 slab, weights, region masks."""
    node_adj = np.asarray(node_adj)
    in_maps = []
    ri = np.arange(P)[:, None]  # [128,1]
    for d in range(NCORES):
        slab = np.zeros((N, SLAB), FP8)
        maskL = np.zeros((P, SLAB), np.float32)
        maskU = np.zeros((P, SLAB), np.float32)
        c0 = CRANGE * d
        for g in range(TPC):
            entries = plan[d][g]
            if not entries:
                continue
            cols = np.array([c for c, _ in entries], np.int64)
            sl = slice(g * W, g * W + len(cols))
            gathered = (node_adj[:, cols] != 0).astype(FP8)
            # rotate rows: local row rho = (abs_row - CRANGE*d) mod N
            slab[:, sl] = np.concatenate([gathered[c0:], gathered[:c0]], axis=0)
            # strict step masks vs the crossing tile's absolute row base;
            # strictness zeroes the diagonal on both sides
            base = c0 + g * P
            maskL[:, sl] = (base + ri < cols[None, :])
            maskU[:, sl] = (base + ri > cols[None, :])
        # region mask packs [P, 16, RW]: for crossing pair q (tiles j=2q,
        # j+1), region = slab cols [jW, (j+2)W) = zones B|C:
        #   primary sub0 (tile j):   [maskL_B | ones_C]
        #   primary sub1 (tile j+1): [ones_B  | maskU_C]
        #   secondary sub0:          [maskU_B | zeros_C]
        #   secondary sub1:          [zeros_B | maskL_C]
        mp = np.zeros((P, 16, RW), FP8)
        for q in range(4):
            j = 2 * q
            zB = slice(j * W, (j + 1) * W)
            zC = slice((j + 1) * W, (j + 2) * W)
            mp[:, 4 * q + 0, 0:W] = maskL[:, zB].astype(FP8)
            mp[:, 4 * q + 0, W:RW] = 1.0
            mp[:, 4 * q + 1, 0:W] = 1.0
            mp[:, 4 * q + 1, W:RW] = maskU[:, zC].astype(FP8)
            mp[:, 4 * q + 2, 0:W] = maskU[:, zB].astype(FP8)
            mp[:, 4 * q + 3, W:RW] = maskL[:, zC].astype(FP8)
        # tile-major DRAM layout in processing order
        adjT = np.zeros((P, NT, SLAB), FP8)
        for p, j in enumerate(ORDER):
            adjT[:, p, :] = slab[j * P : (j + 1) * P, :]
        wm = _build_wmat(wside, d)
        in_maps.append(
            {
                "adj": np.ascontiguousarray(adjT),
                "whead": np.ascontiguousarray(wm[:, 0:4, :]),
                "wmat": wm,
                "masks": np.ascontiguousarray(mp),
            }
        )
    return in_maps


def _side_contrib(x):
    cnt, poscnt = x[0], x[1]
    poslogit = x[2] + x[3]
    sumexp = x[4] + x[5]
    valid = (cnt > 0.5) & (np.abs(poscnt - 1.0) < 0.25)
    lse = np.log(np.where(valid, np.maximum(sumexp, 1e-300), 1.0))
    return np.where(valid, (lse - poslogit) / np.maximum(cnt, 1.0), 0.0)


def _combine(stats_list, plan, overflow, outputs, targets, node_adj):
    """stats_list: per-core [12, SLAB] f32 -> scalar loss (f64 math)."""
    total = 0.0
    for d in range(NCORES):
        x = np.asarray(stats_list[d], np.float64)
        contrib = _side_contrib(x[0:NW]) + _side_contrib(x[NW:M])
        for g in range(TPC):
            for i, (_, m) in enumerate(plan[d][g]):
                total += m * contrib[g * W + i]
    if overflow:
        out = np.asarray(outputs, np.float64).reshape(-1)
        pos = np.asarray(targets).reshape(-1) != 0
        rows = np.arange(N)
        for c, m in overflow:
            col = (np.asarray(node_adj[:, c]).reshape(-1) != 0) & (rows != c)
            for mask in (col & (rows < c), col & (rows >= c)):
                cnt = int(mask.sum())
                pcnt = int((mask & pos).sum())
                if cnt > 0 and pcnt == 1:
                    lse = np.log(np.exp(out[mask]).sum())
                    pl = out[mask & pos].sum()
                    total += m * (lse - pl) / cnt
    return np.float32(total)


def _ensure_axon_hooks_stub():
    """bass_utils imports antenv.axon_hooks when tracing is requested via
    env; the module is absent on some images. Provide a no-op stub so the
    import never crashes (hook=None -> bass_utils skips tracing)."""
    import sys
    import types

    try:
        import antenv.axon_hooks  # noqa: F401
    except ImportError:
        mod = types.ModuleType("antenv.axon_hooks")
        state = {"hook": None}
        mod.set_axon_ntff_profile_hook = lambda h: state.__setitem__("hook", h)
        mod.get_axon_ntff_profile_hook = lambda: state["hook"]
        sys.modules["antenv.axon_hooks"] = mod


def _device_stats(in_maps):
    _ensure_axon_hooks_stub()
    from concourse.bass_utils import run_bass_kernel_spmd

    if "nc" not in _BASS_CACHE:
        _BASS_CACHE["nc"] = _build_bass()
    last_exc = None
    for attempt in range(4):
        try:
            res = run_bass_kernel_spmd(
                _BASS_CACHE["nc"], in_maps, core_ids=list(range(NCORES))
            )
            return [r["stats"] for r in res.results]
        except Exception as e:  # transient NRT/accelerator hiccups
            last_exc = e
            try:
                # a fresh PJRT client usually recovers a transiently
                # "unrecoverable" accelerator; mirrors a process restart
                import jax
                import jax.extend.backend as _jeb

                jax.clear_caches()
                _jeb.clear_backends()
            except Exception:
                pass
            import time

            time.sleep(2.0 * (attempt + 1))
    raise last_exc


def _sim_stats(in_maps):
    """Numpy emulation of the device kernel (same inputs incl. fp8
    quantization and wmat slot packing), for logic + precision validation."""
    outs = []
    for im in in_maps:
        adjT = im["adj"].astype(np.float32)           # [P, NT, SLAB]
        w = im["wmat"].astype(np.float32)             # [P, 2*NSLOT, MP]
        mp = im["masks"].astype(np.float32)           # [P, 16, RW]
        acc = np.zeros((MP, SLAB), np.float32)
        for p2 in range(NPAIR):
            a0, a1 = adjT[:, 2 * p2, :], adjT[:, 2 * p2 + 1, :]
            if p2 in CROSS_PAIRS:
                q = p2 - CROSS_PAIRS[0]
                j = 2 * q
                zA, zC = j * W, (j + 2) * W
                acc[:, :zA] += w[:, 2 * p2].T @ a0[:, :zA]
                acc[:, :zA] += w[:, 2 * p2 + 1].T @ a1[:, :zA]
                reg0, reg1 = a0[:, zA:zC], a1[:, zA:zC]
                acc[:, zA:zC] += w[:, 2 * (36 + q)].T @ (reg0 * mp[:, 4 * q + 0])
                acc[:, zA:zC] += w[:, 2 * (36 + q) + 1].T @ (reg1 * mp[:, 4 * q + 1])
                acc[:, zA:zC] += w[:, 2 * (40 + q)].T @ (reg0 * mp[:, 4 * q + 2])
                acc[:, zA:zC] += w[:, 2 * (40 + q) + 1].T @ (reg1 * mp[:, 4 * q + 3])
                acc[:, zC:] += w[:, 2 * (NPAIR + q)].T @ a0[:, zC:]
                acc[:, zC:] += w[:, 2 * (NPAIR + q) + 1].T @ a1[:, zC:]
            else:
                acc += w[:, 2 * p2].T @ a0
                acc += w[:, 2 * p2 + 1].T @ a1
        outs.append(acc[0:M])
    return outs


def prepare(outputs, targets, node_adj, idx_node):
    wside = _host_weights(outputs, targets)
    plan, overflow = _plan_columns(idx_node)
    in_maps = _build_inputs(node_adj, wside, plan)
    return in_maps, plan, overflow


def kernel(outputs, targets, node_adj, idx_node, _simulate=False):
    in_maps, plan, overflow = prepare(outputs, targets, node_adj, idx_node)
    stats = _sim_stats(in_maps) if _simulate else _device_stats(in_maps)
    return _combine(stats, plan, overflow, outputs, targets, node_adj)
